# revision 1
# baseline (speedup 1.0000x reference)
"""Distributed Trainium2 (8 NeuronCore) kernel for a 2-layer GCN diffusion
denoiser: out = GCN2(relu(LN(GCN1(h + t_emb)))).

Sharding: nodes (and their incident edges) are sharded across the 8 cores by
contiguous dst ranges.  Each core computes x@W for its shard (bf16), the
per-conv feature tables are AllGathered, edge source rows are fetched with
dma_gather on 4 parallel SWDGE queues (one per source-range bucket), and the
scatter-add aggregation is a chain of one-hot matmuls on the TensorEngine
accumulated in PSUM per 128-dst window (the self-loop term is a diagonal
one-hot against the core-local table).  All floating-point math runs on
device; the host only preprocesses the integer edge structure.
"""

import os
import sys
from contextlib import ExitStack

if "/opt/trn_rl_repo" not in sys.path:
    sys.path.insert(0, "/opt/trn_rl_repo")

import numpy as np
import ml_dtypes

import concourse.bacc as bacc
import concourse.bass as bass
import concourse.mybir as mybir
from concourse.bass_utils import run_bass_kernel_spmd
from concourse.library_config import mlp

BF16 = ml_dtypes.bfloat16
F32 = mybir.dt.float32
BF = mybir.dt.bfloat16
I16 = mybir.dt.int16
Alu = mybir.AluOpType
Act = mybir.ActivationFunctionType
AxisX = mybir.AxisListType.X

N_NODES = 100000
C = 128
N_CORES = 8
NBUK = 4

NPS = 4           # rotating PSUM banks for per-window accumulation
NGBQ = 3          # gather-buffer slots per bucket queue
CALL_CHUNKS = 8   # edge-chunks per dma_gather call (<=1024 idxs: HW limit)


# ---------------------------------------------------------------------------
# host-side schedule (pure integer graph preprocessing)
# ---------------------------------------------------------------------------

class _Item:
    __slots__ = ("kind", "w", "q", "pos", "ec", "start", "stop")

    def __init__(self, kind, w, q, pos, ec, start, stop):
        self.kind, self.w, self.q, self.pos, self.ec = kind, w, q, pos, ec
        self.start, self.stop = start, stop


def _make_schedule(src, dst, coef, shard):
    K = N_CORES
    npad = shard * K
    nt = shard // 128
    bukrows = npad // NBUK
    assert bukrows <= 32767 and npad % NBUK == 0 and shard % 128 == 0

    counts = np.zeros((K, NBUK, nt), np.int64)
    per_core_edges = []
    for k in range(K):
        m = (dst >= k * shard) & (dst < (k + 1) * shard)
        s, d, c = src[m], dst[m] - k * shard, coef[m]
        q = s // bukrows
        w = d // 128
        order = np.lexsort((w, q))
        s, d, c, q, w = s[order], d[order], c[order], q[order], w[order]
        np.add.at(counts[k], (q, w), 1)
        per_core_edges.append((s, d, c, q, w))

    nch_qw = -(-counts.max(axis=0) // 128)     # [NBUK, nt] shared chunk counts

    # bucket-major chunk positions: bucket q's stream is its (q, w) runs in
    # ascending w; global ec = buk_base[q] + within-bucket position.
    nchq = nch_qw.sum(axis=1)                  # chunks per bucket
    buk_base = np.zeros(NBUK + 1, np.int64)
    buk_base[1:] = np.cumsum(nchq)
    nech = int(buk_base[-1])
    chunk_pos = {}                             # (q, w) -> within-bucket pos
    for q in range(NBUK):
        p = 0
        for w in range(nt):
            if nch_qw[q, w]:
                chunk_pos[(q, w)] = p
                p += int(nch_qw[q, w])

    # window-major item stream (diag first, then each bucket's chunks)
    items = []
    win_item0 = []
    for w in range(nt):
        win_item0.append(len(items))
        its = [_Item("diag", w, -1, -1, -1, False, False)]
        for q in range(NBUK):
            for j in range(int(nch_qw[q, w])):
                pos = chunk_pos[(q, w)] + j
                its.append(_Item("edge", w, q, pos,
                                 int(buk_base[q]) + pos, False, False))
        its[0].start = True
        its[-1].stop = True
        items.extend(its)
    nitem = len(items)

    # gather calls: per bucket, CALL_CHUNKS chunks per call
    ncall_q = [int(-(-nchq[q] // CALL_CHUNKS)) if nchq[q] else 0
               for q in range(NBUK)]
    call_sizes = [[int(min(CALL_CHUNKS, int(nchq[q]) - j * CALL_CHUNKS))
                   for j in range(ncall_q[q])] for q in range(NBUK)]
    # window containing the last chunk of each (q, call): for WAR release
    pos_to_win = [dict() for _ in range(NBUK)]
    for (q, w), p0 in chunk_pos.items():
        for j in range(int(nch_qw[q, w])):
            pos_to_win[q][p0 + j] = w
    call_rel_win = []
    for q in range(NBUK):
        rels = []
        for j in range(ncall_q[q]):
            last = min((j + 1) * CALL_CHUNKS, int(nchq[q])) - 1
            rels.append(pos_to_win[q][last])
        call_rel_win.append(rels)

    if nt > 1:
        max_items_win = max(
            (win_item0[i + 1] if i + 1 < nt else nitem) - win_item0[i]
            for i in range(nt))
    else:
        max_items_win = nitem

    # per-core padded edge-data arrays, indexed by global ec
    core_arrays = []
    for k in range(K):
        s, d, c, q, w = per_core_edges[k]
        idxl = np.zeros(max(nech, 1) * 128, np.int16)
        dstl = np.zeros(max(nech, 1) * 128, np.float32)
        cf = np.zeros(max(nech, 1) * 128, np.float32)
        keys = q.astype(np.int64) * nt + w.astype(np.int64)
        for (qq, ww), p0 in chunk_pos.items():
            key = qq * nt + ww
            lo = np.searchsorted(keys, key, "left")
            hi = np.searchsorted(keys, key, "right")
            n = hi - lo
            base = (int(buk_base[qq]) + p0) * 128
            idxl[base:base + n] = (s[lo:hi] - qq * bukrows).astype(np.int16)
            dstl[base:base + n] = (d[lo:hi] % 128).astype(np.float32)
            cf[base:base + n] = c[lo:hi]
        gidx = np.tile(idxl.reshape(-1, 16).T, (8, 1)).copy()
        dst2d = dstl.reshape(-1, 128).T.copy()
        cf2d = cf.reshape(-1, 128).T.copy()
        core_arrays.append((gidx, dst2d, cf2d))

    return dict(npad=npad, nt=nt, bukrows=bukrows, items=items,
                win_item0=win_item0, nitem=nitem, nech=nech,
                buk_base=[int(x) for x in buk_base],
                ncall_q=ncall_q, call_sizes=call_sizes,
                call_rel_win=call_rel_win, max_items_win=int(max_items_win),
                core_arrays=core_arrays)


# ---------------------------------------------------------------------------
# bass program
# ---------------------------------------------------------------------------

class _Waits:
    """Per-(engine, sem) monotone tracker; emits wait_ge only when it rises."""

    def __init__(self):
        self.seen = {}

    def __call__(self, eng, s, val):
        key = (id(eng), id(s))
        if val > self.seen.get(key, 0):
            eng.wait_ge(s, val)
            self.seen[key] = val


def _build(S, shard):
    nt, npad, bukrows = S["nt"], S["npad"], S["bukrows"]
    items, win_item0, nitem = S["items"], S["win_item0"], S["nitem"]
    nech, buk_base = S["nech"], S["buk_base"]
    ncall_q, call_sizes = S["ncall_q"], S["call_sizes"]
    call_rel_win = S["call_rel_win"]
    necw = max(nech, 1)
    NOH = min(64, max(16, 2 * S["max_items_win"]))
    PHASE = int(os.environ.get("BASS_PHASE", "5"))

    nc = bacc.Bacc("TRN2", num_swdge_queues=NBUK,
                   detect_race_conditions=not os.environ.get("BASS_NO_RACE"))

    din = lambda n, sh, dt: nc.declare_dram_parameter(n, sh, dt, isOutput=False)
    h_sT_d = din("h_sT", [128, shard], BF)
    gidx_d = din("gidx", [128, necw * 8], I16)
    dst_d = din("dst2d", [128, necw], F32)
    cf_d = din("coef2d", [128, necw], F32)
    dv2_d = din("dinv2col", [128, nt], F32)
    W1b_d = din("W1b", [128, 128], BF)
    W2b_d = din("W2b", [128, 128], BF)
    W1f_d = din("W1f", [128, 128], F32)
    tW2f_d = din("tW2f", [128, 128], F32)
    tW1c_d = din("tW1col", [128, 1], F32)
    tb1c_d = din("tb1col", [128, 1], F32)
    tb2c_d = din("tb2col", [128, 1], F32)
    tcol_d = din("tcol", [128, 1], F32)
    b1r_d = din("b1rep", [128, 128], F32)
    b2r_d = din("b2rep", [128, 128], F32)
    lnwr_d = din("lnwrep", [128, 128], BF)
    lnbr_d = din("lnbrep", [128, 128], BF)
    iota_d = din("iota", [128, 128], BF)
    linc_d = din("lincol", [128, 1], F32)
    eps_d = din("epscol", [128, 1], F32)
    idm_d = din("idmat", [128, 128], BF)
    ones_d = din("onesrow", [1, 128], BF)
    out_d = nc.declare_dram_parameter("out_shard", [shard, 128], F32, isOutput=True)

    ag1_in = nc.dram_tensor("ag1_in", [shard, 128], BF)
    table1 = nc.dram_tensor("table1", [npad, 128], BF, addr_space="Shared")
    ag2_in = nc.dram_tensor("ag2_in", [shard, 128], BF)
    table2 = nc.dram_tensor("table2", [npad, 128], BF, addr_space="Shared")
    tables = [table1, table2]
    ag_ins = [ag1_in, ag2_in]

    with ExitStack() as ctx:
        sbuf = lambda n, sh, dt: ctx.enter_context(nc.sbuf_tensor(n, sh, dt))
        psum = lambda n, sh, dt=F32: ctx.enter_context(nc.psum_tensor(n, sh, dt))
        sem = lambda n: ctx.enter_context(nc.semaphore(n))

        hsT = sbuf("hsT", [128, shard], BF)
        gidx = sbuf("gidx_sb", [128, necw * 8], I16)
        dst2d = sbuf("dst2d_sb", [128, necw], F32)
        cf2d = sbuf("cf2d_sb", [128, necw], F32)
        dv2 = sbuf("dv2_sb", [128, nt], F32)
        W1b = sbuf("W1b_sb", [128, 128], BF)
        W2b = sbuf("W2b_sb", [128, 128], BF)
        W1f = sbuf("W1f_sb", [128, 128], F32)
        tW2f = sbuf("tW2f_sb", [128, 128], F32)
        tW1c = sbuf("tW1c_sb", [128, 1], F32)
        tb1c = sbuf("tb1c_sb", [128, 1], F32)
        tb2c = sbuf("tb2c_sb", [128, 1], F32)
        tcol = sbuf("tcol_sb", [128, 1], F32)
        b1r = sbuf("b1r_sb", [128, 128], F32)
        b2r = sbuf("b2r_sb", [128, 128], F32)
        lnwr = sbuf("lnwr_sb", [128, 128], BF)
        lnbr = sbuf("lnbr_sb", [128, 128], BF)
        iota = sbuf("iota_sb", [128, 128], BF)
        linc = sbuf("linc_sb", [128, 1], F32)
        epsc = sbuf("eps_sb", [128, 1], F32)
        idmat = sbuf("idmat_sb", [128, 128], BF)
        onesr = sbuf("ones_sb", [1, 128], BF)

        xw1 = sbuf("xw1", [128, shard], BF)
        xw2 = sbuf("xw2", [128, shard], BF)
        agg = sbuf("agg", [128, shard], F32)
        h2full = sbuf("h2full", [128, shard], BF)
        gb = sbuf("gb", [128, NBUK, NGBQ, CALL_CHUNKS * 128], BF)
        oh = sbuf("oh", [128, NOH, 128], BF)
        h2T = sbuf("h2T", [128, 4, 128], BF)
        c16 = sbuf("c16", [128, 2, 128], BF)
        sqscr = sbuf("sqscr", [128, 128], F32)
        ucol = sbuf("ucol", [128, 1], F32)
        vcol = sbuf("vcol", [128, 1], F32)
        r1bf = sbuf("r1bf", [1, 128], BF)
        stat = sbuf("stat", [128, 8], F32)

        ps_run = [psum(f"ps_run{i}", [128, 128]) for i in range(NPS)]
        ps_pa = [psum("ps_pa0", [128, 128]), psum("ps_pa1", [128, 128])]
        ps_tr = [psum("ps_tr0", [128, 128], BF), psum("ps_tr1", [128, 128], BF)]

        s_ld = sem("s_ld")
        s_tb1 = sem("s_tb1")
        s_tb2 = sem("s_tb2")
        s_cc = sem("s_cc")
        s_gq = [sem(f"s_gq{q}") for q in range(NBUK)]
        s_out = sem("s_out")
        s_pe_run = sem("s_pe_run")     # one inc per completed window
        s_pe_pa = sem("s_pe_pa")
        s_pe_pa2 = sem("s_pe_pa2")
        s_pe_tm = sem("s_pe_tm")
        s_pe_tr = sem("s_pe_tr")
        s_dv_oh = sem("s_dv_oh")
        s_dv_drain = sem("s_dv_drain")  # one inc per drained window
        s_dv_pa = sem("s_dv_pa")
        s_dv_pa2 = sem("s_dv_pa2")
        s_dv_tm = sem("s_dv_tm")
        s_dv_trc = sem("s_dv_trc")
        s_dv_ep = sem("s_dv_ep")
        s_ac = sem("s_ac")
        s_ac_h2 = sem("s_ac_h2")

        wt = _Waits()
        N_LOADS = 22

        with nc.Block() as block:

            # ---------------- SYNC ----------------
            @block.sync
            def _(sync):
                loads = [
                    (hsT, h_sT_d), (gidx, gidx_d), (dst2d, dst_d), (cf2d, cf_d),
                    (dv2, dv2_d), (W1b, W1b_d), (W2b, W2b_d), (W1f, W1f_d),
                    (tW2f, tW2f_d), (tW1c, tW1c_d), (tb1c, tb1c_d),
                    (tb2c, tb2c_d), (tcol, tcol_d), (b1r, b1r_d), (b2r, b2r_d),
                    (lnwr, lnwr_d), (lnbr, lnbr_d), (iota, iota_d),
                    (linc, linc_d), (epsc, eps_d), (idmat, idm_d),
                    (onesr, ones_d),
                ]
                assert len(loads) == N_LOADS
                for dst_t, src_t in loads:
                    sync.dma_start(dst_t[:, :], src_t[:, :]).then_inc(s_ld, 16)

                for t in range(nt):
                    wt(sync, s_dv_pa, t + 1)
                    sync.dma_start(ag1_in[t * 128:(t + 1) * 128, :],
                                   xw1[:, t * 128:(t + 1) * 128]).then_inc(s_tb1, 16)

                if PHASE >= 4:
                    for w in range(nt):
                        wt(sync, s_dv_pa2, w + 1)
                        sync.dma_start(ag2_in[w * 128:(w + 1) * 128, :],
                                       xw2[:, w * 128:(w + 1) * 128]).then_inc(
                                           s_tb2, 16)

                for w in range(nt):
                    if PHASE == 1:
                        wt(sync, s_dv_pa, nt)
                    elif PHASE <= 4:
                        wt(sync, s_dv_drain, w + 1)
                    else:
                        wt(sync, s_dv_drain, nt + w + 1)
                    sync.dma_start(out_d[w * 128:(w + 1) * 128, :],
                                   agg[:, w * 128:(w + 1) * 128]).then_inc(s_out, 16)
                wt(sync, s_out, 16 * nt)

            # ---------------- GPSIMD ----------------
            @block.gpsimd
            def _(gpsimd):
                gpsimd.load_library(mlp)
                NCONV = 0 if PHASE == 1 else (1 if PHASE <= 4 else 2)
                for conv in range(NCONV):
                    wt(gpsimd, s_tb1 if conv == 0 else s_tb2, 16 * nt)
                    gpsimd.collective_compute(
                        "AllGather", Alu.bypass,
                        replica_groups=[list(range(N_CORES))],
                        ins=[ag_ins[conv].ap().opt()],
                        outs=[tables[conv].ap().opt()]).then_inc(s_cc, 1)
                    wt(gpsimd, s_cc, conv + 1)
                    maxcall = max(ncall_q) if ncall_q else 0
                    for j in range(maxcall):
                        for q in range(NBUK):
                            if j >= ncall_q[q]:
                                continue
                            if j >= NGBQ:
                                rel_w = call_rel_win[q][j - NGBQ]
                                wt(gpsimd, s_pe_run, conv * nt + rel_w + 1)
                            nchk = call_sizes[q][j]
                            nidx = nchk * 128
                            ec0 = buk_base[q] + j * CALL_CHUNKS
                            dstap = gb[:, q, j % NGBQ, 0:nchk * 128].rearrange(
                                "p (n e) -> p n e", e=128)
                            gpsimd.dma_gather(
                                dstap,
                                tables[conv][q * bukrows:(q + 1) * bukrows, :],
                                gidx[:, ec0 * 8:(ec0 + nchk) * 8], nidx, nidx,
                                128, single_packet=True, queue_num=q,
                            ).then_inc(s_gq[q], 16)
                for q in range(NBUK):
                    if ncall_q[q]:
                        wt(gpsimd, s_gq[q],
                           16 * (0 if PHASE == 1 else
                                 (1 if PHASE <= 4 else 2)) * ncall_q[q])

            # ---------------- TENSOR ----------------
            @block.tensor
            def _(tensor):
                wt(tensor, s_ld, 16 * N_LOADS)
                # t-MLP
                wt(tensor, s_dv_tm, 1)
                tensor.matmul(ps_pa[0][:, 0:1], tW2f[:, :], ucol[:, :],
                              start=True, stop=True).then_inc(s_pe_tm, 1)
                wt(tensor, s_dv_tm, 2)
                tensor.matmul(ps_pa[1][0:1, 0:128], vcol[:, :], W1f[:, :],
                              start=True, stop=True).then_inc(s_pe_tm, 1)
                wt(tensor, s_dv_tm, 3)
                # conv1 phase A
                for t in range(nt):
                    if t >= 2:
                        wt(tensor, s_dv_pa, t - 1)
                    p = ps_pa[t % 2]
                    tensor.matmul(p[:, :], hsT[:, t * 128:(t + 1) * 128], W1b[:, :],
                                  start=True, stop=False)
                    tensor.matmul(p[:, :], onesr[:, :], r1bf[:, :],
                                  start=False, stop=True).then_inc(s_pe_pa, 1)

                for conv in range(0 if PHASE == 1 else 1 if PHASE <= 4 else 2):
                    wt(tensor, s_dv_pa if conv == 0 else s_dv_pa2, nt)
                    xw = xw1 if conv == 0 else xw2
                    for w in range(nt):
                        W = conv * nt + w
                        if W >= NPS:
                            wt(tensor, s_dv_drain, W - NPS + 1)
                        i0 = win_item0[w]
                        i1 = win_item0[w + 1] if w + 1 < nt else nitem
                        wt(tensor, s_dv_oh, conv * nitem + i1)
                        for i in range(i0, i1):
                            it = items[i]
                            if it.kind == "edge":
                                wt(tensor, s_gq[it.q],
                                   16 * (conv * ncall_q[it.q] +
                                         it.pos // CALL_CHUNKS + 1))
                            lhs = oh[:, (conv * nitem + i) % NOH, :]
                            if it.kind == "diag":
                                rhs = xw[:, w * 128:(w + 1) * 128]
                            else:
                                slot = (it.pos // CALL_CHUNKS) % NGBQ
                                off = it.pos % CALL_CHUNKS
                                rhs = gb[:, it.q, slot, off * 128:(off + 1) * 128]
                            mm = tensor.matmul(ps_run[W % NPS][:, :], lhs, rhs,
                                               start=it.start, stop=it.stop)
                            if it.stop:
                                mm.then_inc(s_pe_run, 1)
                    if conv == 0 and PHASE >= 4:
                        def a2_mm(e):
                            wt(tensor, s_dv_trc, e + 1)
                            if e >= 2:
                                wt(tensor, s_dv_pa2, e - 1)
                            tensor.matmul(ps_pa[e % 2][:, :], h2T[:, e % 4, :],
                                          W2b[:, :], start=True,
                                          stop=True).then_inc(s_pe_pa2, 1)

                        for e in range(nt):
                            wt(tensor, s_ac_h2, e + 1)
                            if e >= 2:
                                wt(tensor, s_dv_trc, e - 1)
                            tensor.transpose(ps_tr[e % 2][:, :],
                                             h2full[:, e * 128:(e + 1) * 128],
                                             idmat[:, :]).then_inc(s_pe_tr, 1)
                            if e >= 1:
                                a2_mm(e - 1)
                        if nt >= 1:
                            a2_mm(nt - 1)

            # ---------------- VECTOR ----------------
            def emit_epilogue(vector, w):
                aggw = agg[:, w * 128:(w + 1) * 128]
                ssum, ssq = stat[:, 0:1], stat[:, 1:2]
                smu, ssmu = stat[:, 2:3], stat[:, 3:4]
                svarn, ssd, srstd = stat[:, 4:5], stat[:, 5:6], stat[:, 6:7]
                vector.drain()
                vector.tensor_reduce(ssum, aggw, AxisX, Alu.add)
                vector.tensor_mul(sqscr[:, :], aggw, aggw)
                vector.drain()
                vector.tensor_reduce(ssq, sqscr[:, :], AxisX, Alu.add)
                vector.drain()
                vector.tensor_scalar(smu, ssum, 1.0 / 128.0, None, Alu.mult)
                vector.drain()
                vector.tensor_scalar(ssmu, ssum, smu, None, Alu.mult)
                vector.drain()
                vector.tensor_scalar(svarn, ssq, ssmu, 1.0 / 128.0,
                                     Alu.subtract, Alu.mult).then_inc(s_dv_ep, 1)
                wt(vector, s_ac, w + 1)
                vector.reciprocal(srstd, ssd)
                vector.drain()
                if w >= 2:
                    wt(vector, s_ac_h2, w - 1)
                cw = c16[:, w % 2, :]
                vector.tensor_scalar(cw, aggw, smu, srstd, Alu.subtract, Alu.mult)
                vector.drain()
                vector.tensor_mul(cw, cw, lnwr[:, :])
                vector.drain()
                vector.tensor_add(cw, cw, lnbr[:, :]).then_inc(s_dv_ep, 1)

            @block.vector
            def _(vector):
                PH = PHASE
                wt(vector, s_ld, 16 * N_LOADS)
                # t-MLP
                vector.tensor_scalar(ucol[:, :], tW1c[:, :], tcol[:, :], tb1c[:, :],
                                     Alu.mult, Alu.add)
                vector.drain()
                vector.tensor_relu(ucol[:, :], ucol[:, :]).then_inc(s_dv_tm, 1)
                wt(vector, s_pe_tm, 1)
                vector.tensor_add(vcol[:, :], ps_pa[0][:, 0:1],
                                  tb2c[:, :]).then_inc(s_dv_tm, 1)
                wt(vector, s_pe_tm, 2)
                vector.tensor_copy(r1bf[:, :],
                                   ps_pa[1][0:1, 0:128]).then_inc(s_dv_tm, 1)
                # conv1 phase-A PSUM -> SBUF (bf16)
                for t in range(nt):
                    wt(vector, s_pe_pa, t + 1)
                    vector.tensor_copy(xw1[:, t * 128:(t + 1) * 128],
                                       ps_pa[t % 2][:, :]).then_inc(s_dv_pa, 1)

                def drain_win(vector, conv, dw, brep):
                    D = conv * nt + dw
                    aggw = agg[:, dw * 128:(dw + 1) * 128]
                    wt(vector, s_pe_run, D + 1)
                    vector.tensor_add(aggw, brep[:, :],
                                      ps_run[D % NPS][:, :]).then_inc(s_dv_drain, 1)
                    if conv == 0 and PH >= 3:
                        emit_epilogue(vector, dw)

                for conv in range(0 if PH == 1 else 1 if PH <= 4 else 2):
                    brep = b1r if conv == 0 else b2r
                    for w in range(nt):
                        i0 = win_item0[w]
                        i1 = win_item0[w + 1] if w + 1 < nt else nitem
                        for i in range(i0, i1):
                            gi = conv * nitem + i
                            it = items[i]
                            if gi >= NOH:
                                ii = gi - NOH
                                blk = (ii // nitem) * nt + items[ii % nitem].w
                                wt(vector, s_pe_run, blk + 1)
                            if it.kind == "diag":
                                s1, s2 = linc[:, :], dv2[:, w:w + 1]
                            else:
                                s1 = dst2d[:, it.ec:it.ec + 1]
                                s2 = cf2d[:, it.ec:it.ec + 1]
                            vector.tensor_scalar(oh[:, gi % NOH, :], iota[:, :],
                                                 s1, s2, Alu.is_equal,
                                                 Alu.mult).then_inc(s_dv_oh, 1)
                        if w >= 1:
                            drain_win(vector, conv, w - 1, brep)
                    drain_win(vector, conv, nt - 1, brep)
                    if conv == 0 and PH >= 4:
                        def pa2_copy(e):
                            wt(vector, s_pe_pa2, e + 1)
                            vector.tensor_copy(xw2[:, e * 128:(e + 1) * 128],
                                               ps_pa[e % 2][:, :]).then_inc(
                                                   s_dv_pa2, 1)

                        for e in range(nt):
                            wt(vector, s_pe_tr, e + 1)
                            if e >= 4:
                                wt(vector, s_pe_pa2, e - 3)
                            vector.tensor_copy(h2T[:, e % 4, :],
                                               ps_tr[e % 2][:, :]).then_inc(
                                                   s_dv_trc, 1)
                            if e >= 1:
                                pa2_copy(e - 1)
                        if nt >= 1:
                            pa2_copy(nt - 1)

            # ---------------- SCALAR (ACT) ----------------
            @block.scalar
            def _(scalar):
                wt(scalar, s_ld, 16 * N_LOADS)
                for e in range(nt if PHASE >= 3 else 0):
                    wt(scalar, s_dv_ep, 2 * e + 1)
                    scalar.activation(stat[:, 5:6], stat[:, 4:5], Act.Sqrt,
                                      bias=epsc[:, :]).then_inc(s_ac, 1)
                    wt(scalar, s_dv_ep, 2 * e + 2)
                    scalar.activation(h2full[:, e * 128:(e + 1) * 128],
                                      c16[:, e % 2, :],
                                      Act.Relu).then_inc(s_ac_h2, 1)

        nc.compile()
    return nc


# ---------------------------------------------------------------------------
# top level
# ---------------------------------------------------------------------------

LAST_NC = None


def _run_problem(h_noisy, edge_index, t, tW1, tb1, tW2, tb2, W1, b1, W2, b2,
                 ln_w, ln_b, n_nodes, shard, trace_dir=None):
    K = N_CORES
    npad = shard * K
    src = np.asarray(edge_index[0], np.int64)
    dst = np.asarray(edge_index[1], np.int64)

    deg = (np.bincount(dst, minlength=n_nodes).astype(np.float32) + 1.0)
    dinv = (1.0 / np.sqrt(deg)).astype(np.float32)
    coef = (dinv[src] * dinv[dst]).astype(np.float32)
    dinv2 = (dinv * dinv).astype(np.float32)
    dinv2_pad = np.ones(npad, np.float32)
    dinv2_pad[:n_nodes] = dinv2

    S = _make_schedule(src, dst, coef, shard)
    nt = S["nt"]

    h_pad = np.zeros((npad, C), np.float32)
    h_pad[:n_nodes] = np.asarray(h_noisy, np.float32)

    shared = {
        "W1b": np.asarray(W1, np.float32).astype(BF16),
        "W2b": np.asarray(W2, np.float32).astype(BF16),
        "W1f": np.asarray(W1, np.float32),
        "tW2f": np.asarray(tW2, np.float32),
        "tW1col": np.asarray(tW1, np.float32).reshape(C, 1),
        "tb1col": np.asarray(tb1, np.float32).reshape(C, 1),
        "tb2col": np.asarray(tb2, np.float32).reshape(C, 1),
        "tcol": np.full((C, 1), np.float32(np.asarray(t).reshape(-1)[0]), np.float32),
        "b1rep": np.tile(np.asarray(b1, np.float32).reshape(1, C), (128, 1)),
        "b2rep": np.tile(np.asarray(b2, np.float32).reshape(1, C), (128, 1)),
        "lnwrep": np.tile(np.asarray(ln_w, np.float32).reshape(1, C),
                          (128, 1)).astype(BF16),
        "lnbrep": np.tile(np.asarray(ln_b, np.float32).reshape(1, C),
                          (128, 1)).astype(BF16),
        "iota": np.tile(np.arange(128, dtype=np.float32), (128, 1)).astype(BF16),
        "lincol": np.arange(128, dtype=np.float32).reshape(128, 1),
        "epscol": np.full((128, 1), 1e-5, np.float32),
        "idmat": np.eye(128, dtype=np.float32).astype(BF16),
        "onesrow": np.ones((1, 128), np.float32).astype(BF16),
    }

    in_maps = []
    for k in range(K):
        gidx, dst2d, cf2d = S["core_arrays"][k]
        hs = h_pad[k * shard:(k + 1) * shard].astype(BF16)
        dv2col = np.zeros((128, nt), np.float32)
        for w in range(nt):
            dv2col[:, w] = dinv2_pad[k * shard + w * 128: k * shard + (w + 1) * 128]
        m = dict(shared)
        m["h_sT"] = np.ascontiguousarray(hs.T)
        m["gidx"] = gidx
        m["dst2d"] = dst2d
        m["coef2d"] = cf2d
        m["dinv2col"] = dv2col
        in_maps.append(m)

    nc = _build(S, shard)
    global LAST_NC
    LAST_NC = nc

    if trace_dir is not None:
        res = _run_traced(nc, in_maps, trace_dir)
    else:
        res = run_bass_kernel_spmd(nc, in_maps, list(range(K)))

    out = np.concatenate([res.results[k]["out_shard"] for k in range(K)], axis=0)
    return out[:n_nodes].astype(np.float32)


def _run_traced(nc, in_maps, trace_dir):
    """Run with NRT/NTFF profiling via the axon ctypes hook (test harness)."""
    import types
    import antenv
    if "antenv.axon_hooks" not in sys.modules:
        mod = types.ModuleType("antenv.axon_hooks")
        mod._hook = None
        mod.set_axon_ntff_profile_hook = lambda h: setattr(mod, "_hook", h)
        mod.get_axon_ntff_profile_hook = lambda: mod._hook
        sys.modules["antenv.axon_hooks"] = mod
        antenv.axon_hooks = mod
    from trn_agent_boot.trn_boot import _ntff_profile_via_ctypes
    hook = _ntff_profile_via_ctypes("/opt/axon/libaxon_pjrt.so")
    os.makedirs(trace_dir, exist_ok=True)
    with hook(trace_dir, [0]):
        res = run_bass_kernel_spmd(nc, in_maps, list(range(N_CORES)))
    return res


def kernel(h_noisy, edge_index, t, tW1, tb1, tW2, tb2, W1, b1, W2, b2,
           ln_w, ln_b):
    trace_dir = os.environ.get("BASS_KERNEL_TRACE_DIR") or None
    return _run_problem(
        np.asarray(h_noisy), np.asarray(edge_index), np.asarray(t),
        np.asarray(tW1), np.asarray(tb1), np.asarray(tW2), np.asarray(tb2),
        np.asarray(W1), np.asarray(b1), np.asarray(W2), np.asarray(b2),
        np.asarray(ln_w), np.asarray(ln_b),
        n_nodes=N_NODES, shard=12544, trace_dir=trace_dir)



# revision 14
# speedup vs baseline: 2.5278x; 2.5278x over previous
"""Distributed Trainium2 (8 NeuronCore) kernel for a 2-layer GCN diffusion
denoiser: out = GCN2(relu(LN(GCN1(h + t_emb)))).

v2 design:
- Nodes sharded by contiguous dst ranges (12544/core).  Per conv, each core
  computes table rows dinv[n] * (x @ W) (phase A), AllGathers the bf16 table,
  then aggregates per 128-dst window on the TensorEngine: the scatter-add is
  a chain of matmuls whose lhsT are PURE 0/1 one-hot matrices precomputed on
  the host and STREAMED from DRAM (no vector-engine one-hot generation).
- coef = dinv[src]*dinv[dst] is factorized: dinv[src] is folded into the
  table rows, dinv[dst] into the PSUM drain (tensor_scalar mult), so the
  one-hots carry no float payload.  The self-loop term is an identity-matmul
  against the local table slice (dinv^2/dinv = dinv).
- LayerNorm stats are batched per 14-window tile with 3D tensor_reduce; the
  per-window normalize (z) and the ReLU run on the otherwise idle scalar
  engine.  Gathers use 4 SWDGE queues with <=1024 indices per call.
"""

import os
import sys
from contextlib import ExitStack

if "/opt/trn_rl_repo" not in sys.path:
    sys.path.insert(0, "/opt/trn_rl_repo")

import numpy as np
import ml_dtypes

import concourse.bacc as bacc
import concourse.mybir as mybir
from concourse.bass_utils import run_bass_kernel_spmd
from concourse.library_config import mlp

BF16 = ml_dtypes.bfloat16
F32 = mybir.dt.float32
BF = mybir.dt.bfloat16
I16 = mybir.dt.int16
Alu = mybir.AluOpType
Act = mybir.ActivationFunctionType
AxisX = mybir.AxisListType.X

N_NODES = 100000
C = 128
N_CORES = 8
NBUK = 4

NPS = 4           # rotating PSUM banks for per-window accumulation
NGBQ = 2          # gather-buffer slots per bucket queue
if os.environ.get("KDBG_BIGBUF"):
    NGBQ = 8
CALL_CHUNKS = 8   # edge-chunks per dma_gather call (<=1024 idxs: HW limit)
GRAN = 16         # one-hot chunks per streamed DMA granule
NGRBUF = 4        # granule slots resident in SBUF
if os.environ.get("KDBG_BIGBUF"):
    NGRBUF = 16
TS = 14           # LayerNorm tile size (windows per stats batch)
NOUTS = 2         # rotating out-staging buffers (TS windows each)


# ---------------------------------------------------------------------------
# host-side schedule (pure integer graph preprocessing)
# ---------------------------------------------------------------------------

class _Item:
    __slots__ = ("w", "q", "pos", "ec")

    def __init__(self, w, q, pos, ec):
        self.w, self.q, self.pos, self.ec = w, q, pos, ec


def _make_schedule(src, dst, shard):
    K = N_CORES
    npad = shard * K
    nt = shard // 128
    bukrows = npad // NBUK
    assert bukrows <= 32767 and npad % NBUK == 0 and shard % 128 == 0

    counts = np.zeros((K, NBUK, nt), np.int64)
    per_core_edges = []
    for k in range(K):
        m = (dst >= k * shard) & (dst < (k + 1) * shard)
        s, d = src[m], dst[m] - k * shard
        q = s // bukrows
        w = d // 128
        order = np.lexsort((w, q))
        s, d, q, w = s[order], d[order], q[order], w[order]
        np.add.at(counts[k], (q, w), 1)
        per_core_edges.append((s, d, q, w))

    nch_qw = -(-counts.max(axis=0) // 128)     # [NBUK, nt] shared chunk counts

    nchq = nch_qw.sum(axis=1)                  # chunks per bucket
    buk_base = np.zeros(NBUK + 1, np.int64)
    buk_base[1:] = np.cumsum(nchq)
    nech = int(buk_base[-1])
    chunk_pos = {}                             # (q, w) -> within-bucket pos
    for q in range(NBUK):
        p = 0
        for w in range(nt):
            if nch_qw[q, w]:
                chunk_pos[(q, w)] = p
                p += int(nch_qw[q, w])

    # window-major item stream (edge chunks only; self-loop is an identity mm)
    items = []
    win_item0 = []
    for w in range(nt):
        win_item0.append(len(items))
        for q in range(NBUK):
            for j in range(int(nch_qw[q, w])):
                pos = chunk_pos[(q, w)] + j
                items.append(_Item(w, q, pos, int(buk_base[q]) + pos))
    nitem = len(items)
    assert nitem == nech

    # gather calls: per bucket, CALL_CHUNKS chunks per call
    ncall_q = [int(-(-nchq[q] // CALL_CHUNKS)) if nchq[q] else 0
               for q in range(NBUK)]
    call_sizes = [[int(min(CALL_CHUNKS, int(nchq[q]) - j * CALL_CHUNKS))
                   for j in range(ncall_q[q])] for q in range(NBUK)]
    pos_to_win = [dict() for _ in range(NBUK)]
    for (q, w), p0 in chunk_pos.items():
        for j in range(int(nch_qw[q, w])):
            pos_to_win[q][p0 + j] = w
    call_rel_win = []
    for q in range(NBUK):
        rels = []
        for j in range(ncall_q[q]):
            last = min((j + 1) * CALL_CHUNKS, int(nchq[q])) - 1
            rels.append(pos_to_win[q][last])
        call_rel_win.append(rels)

    item_win = np.array([it.w for it in items], np.int64)

    # per-core padded edge-data arrays, indexed by global ec
    core_arrays = []
    for k in range(K):
        s, d, q, w = per_core_edges[k]
        idxl = np.zeros(max(nech, 1) * 128, np.int16)
        ohdst = np.full(max(nech, 1) * 128, -1, np.int64)   # -1 = pad
        keys = q.astype(np.int64) * nt + w.astype(np.int64)
        for (qq, ww), p0 in chunk_pos.items():
            key = qq * nt + ww
            lo = np.searchsorted(keys, key, "left")
            hi = np.searchsorted(keys, key, "right")
            n = hi - lo
            base = (int(buk_base[qq]) + p0) * 128
            idxl[base:base + n] = (s[lo:hi] - qq * bukrows).astype(np.int16)
            ohdst[base:base + n] = d[lo:hi] % 128
        gidx = np.tile(idxl.reshape(-1, 16).T, (8, 1)).copy()
        # oh [128 partitions = edge slot, nech*128 cols]: 1.0 at dst lane.
        # Built per-chunk (ec), then permuted to ITEM order — the device
        # streams oh granules sequentially in item (window-major) order.
        oh = np.zeros((128, max(nech, 1) * 128), np.float32)
        eidx = np.arange(max(nech, 1) * 128)
        valid = ohdst >= 0
        ev = eidx[valid]
        oh[ev % 128, (ev // 128) * 128 + ohdst[valid]] = 1.0
        perm = np.array([it.ec for it in items], np.int64)
        if len(perm):
            oh = np.ascontiguousarray(
                oh.reshape(128, max(nech, 1), 128)[:, perm, :]
            ).reshape(128, max(nech, 1) * 128)
        core_arrays.append((gidx, oh.astype(BF16)))

    return dict(npad=npad, nt=nt, bukrows=bukrows, items=items,
                win_item0=win_item0, nitem=nitem, nech=nech,
                buk_base=[int(x) for x in buk_base],
                ncall_q=ncall_q, call_sizes=call_sizes,
                call_rel_win=call_rel_win, item_win=item_win,
                core_arrays=core_arrays)


# ---------------------------------------------------------------------------
# bass program
# ---------------------------------------------------------------------------

class _Waits:
    """Per-(engine, sem) monotone tracker; emits wait_ge only when it rises."""

    def __init__(self):
        self.seen = {}

    def __call__(self, eng, s, val):
        key = (id(eng), id(s))
        if val > self.seen.get(key, 0):
            eng.wait_ge(s, val)
            self.seen[key] = val


def _build(S, shard, ln_trivial, bias1_zero, bias2_zero):
    nt, npad, bukrows = S["nt"], S["npad"], S["bukrows"]
    items, win_item0, nitem = S["items"], S["win_item0"], S["nitem"]
    nech, buk_base = S["nech"], S["buk_base"]
    ncall_q, call_sizes = S["ncall_q"], S["call_sizes"]
    call_rel_win = S["call_rel_win"]
    necw = max(nech, 1)
    assert nt % TS == 0
    NTILE = nt // TS
    NGRAN = -(-nitem // GRAN)                # granules per conv

    # granule (conv-local g) -> window of its last item
    gran_last_win = [int(S["item_win"][min((g + 1) * GRAN, nitem) - 1])
                     for g in range(NGRAN)]

    nc = bacc.Bacc("TRN2", num_swdge_queues=NBUK,
                   detect_race_conditions=not os.environ.get("BASS_NO_RACE"))

    din = lambda n, sh, dt: nc.declare_dram_parameter(n, sh, dt, isOutput=False)
    h_sT_d = din("h_sT", [128, shard], BF)
    gidx_d = din("gidx", [128, necw * 8], I16)
    oh_d = din("ohmat", [128, necw * 128], BF)
    dinv_d = din("dinvcol", [128, nt], F32)
    W1b_d = din("W1b", [128, 128], BF)
    W2b_d = din("W2b", [128, 128], BF)
    W1f_d = din("W1f", [128, 128], F32)
    tW2f_d = din("tW2f", [128, 128], F32)
    tW1c_d = din("tW1col", [128, 1], F32)
    tb1c_d = din("tb1col", [128, 1], F32)
    tb2c_d = din("tb2col", [128, 1], F32)
    tcol_d = din("tcol", [128, 1], F32)
    eps_d = din("epscol", [128, 1], F32)
    idm_d = din("idmat", [128, 128], BF)
    ones_d = din("onesrow", [1, 128], BF)
    sd_d = din("sqdegrow", [1, shard], BF)
    b1r_d = din("b1row", [1, 128], BF)
    b2r_d = din("b2row", [1, 128], BF)
    lnw_d = din("lnwrep", [128, TS * 128], BF)
    lnb_d = din("lnbrep", [128, TS * 128], BF)
    out_d = nc.declare_dram_parameter("out_shard", [shard, 128], F32, isOutput=True)

    ag1_in = nc.dram_tensor("ag1_in", [shard, 128], BF)
    table1 = nc.dram_tensor("table1", [npad, 128], BF, addr_space="Shared")
    ag2_in = nc.dram_tensor("ag2_in", [shard, 128], BF)
    table2 = nc.dram_tensor("table2", [npad, 128], BF, addr_space="Shared")
    tables = [table1, table2]
    ag_ins = [ag1_in, ag2_in]

    with ExitStack() as ctx:
        sbuf = lambda n, sh, dt: ctx.enter_context(nc.sbuf_tensor(n, sh, dt))
        psum = lambda n, sh, dt=F32: ctx.enter_context(nc.psum_tensor(n, sh, dt))
        sem = lambda n: ctx.enter_context(nc.semaphore(n))

        hsT = sbuf("hsT", [128, shard], BF)
        gidx = sbuf("gidx_sb", [128, necw * 8], I16)
        dinv = sbuf("dinv_sb", [128, nt], F32)
        W1b = sbuf("W1b_sb", [128, 128], BF)
        W2b = sbuf("W2b_sb", [128, 128], BF)
        W1f = sbuf("W1f_sb", [128, 128], F32)
        tW2f = sbuf("tW2f_sb", [128, 128], F32)
        tW1c = sbuf("tW1c_sb", [128, 1], F32)
        tb1c = sbuf("tb1c_sb", [128, 1], F32)
        tb2c = sbuf("tb2c_sb", [128, 1], F32)
        tcol = sbuf("tcol_sb", [128, 1], F32)
        epsc = sbuf("eps_sb", [128, 1], F32)
        idmat = sbuf("idmat_sb", [128, 128], BF)
        onesr = sbuf("ones_sb", [1, 128], BF)
        sdrow = sbuf("sdrow_sb", [1, shard], BF)
        b1row = sbuf("b1row_sb", [1, 128], BF)
        b2row = sbuf("b2row_sb", [1, 128], BF)
        lnwr = lnbr = None
        if not ln_trivial:
            lnwr = sbuf("lnwr_sb", [128, TS * 128], BF)
            lnbr = sbuf("lnbr_sb", [128, TS * 128], BF)

        xw1 = sbuf("xw1", [128, shard], BF)       # table1 rows; later h2
        xw2 = sbuf("xw2", [128, shard], BF)       # table2 rows
        aggb = sbuf("aggb", [128, shard], BF)     # conv1 output (bf16)
        gb = sbuf("gb", [128, NBUK, NGBQ, CALL_CHUNKS * 128], BF)
        oh = sbuf("oh", [128, NGRBUF * GRAN, 128], BF)
        h2T = sbuf("h2T", [128, 4, 128], BF)
        outst = sbuf("outst", [128, NOUTS, TS * 128], F32)
        sqscr = sbuf("sqscr", [128, TS * 128], F32)
        ucol = sbuf("ucol", [128, 1], F32)
        vcol = sbuf("vcol", [128, 1], F32)
        r1bf = sbuf("r1bf", [1, 128], BF)
        ssum = sbuf("ssum", [128, nt], F32)
        ssq = sbuf("ssq", [128, nt], F32)
        nmu = sbuf("nmu", [128, nt], F32)
        varn = sbuf("varn", [128, nt], F32)
        sdev = sbuf("sdev", [128, nt], F32)
        sq14 = sbuf("sq14", [128, nt], F32)
        rstd = sbuf("rstd", [128, nt], F32)
        nmurstd = sbuf("nmurstd", [128, nt], F32)

        ps_run = [psum(f"ps_run{i}", [128, 128]) for i in range(NPS)]
        ps_pa = [psum("ps_pa0", [128, 128]), psum("ps_pa1", [128, 128])]
        ps_tr = [psum("ps_tr0", [128, 128], BF), psum("ps_tr1", [128, 128], BF)]

        s_ld = sem("s_ld")
        s_tb1 = sem("s_tb1")
        s_tb2 = sem("s_tb2")
        s_cc = sem("s_cc")
        s_gq = [sem(f"s_gq{q}") for q in range(NBUK)]
        s_ohs = [sem(f"s_ohs{i}") for i in range(NGRBUF)]
        s_outs = [sem(f"s_outs{i}") for i in range(NOUTS)]
        s_pe_run = sem("s_pe_run")  # one inc per completed window (both convs)
        s_pe_pa = sem("s_pe_pa")
        s_pe_pa2 = sem("s_pe_pa2")
        s_pe_tm = sem("s_pe_tm")
        s_pe_tr = sem("s_pe_tr")
        s_dv_drain = sem("s_dv_drain")  # one inc per drained window (both convs)
        s_dv_pa = sem("s_dv_pa")
        s_dv_pa2 = sem("s_dv_pa2")
        s_dv_tm = sem("s_dv_tm")
        s_dv_trc = sem("s_dv_trc")
        s_dv_st = sem("s_dv_st")    # stats ready (per tile)
        s_ac_sq = sem("s_ac_sq")    # scalar sqrt done (per tile)
        s_dv_rs = sem("s_dv_rs")    # rstd+nmurstd ready (per tile)
        s_ac_z = sem("s_ac_z")      # scalar z done (per window)
        s_ac_h2 = sem("s_ac_h2")    # scalar relu done (per tile)

        wt = _Waits()

        loads_sync = [
            (hsT, h_sT_d), (gidx, gidx_d), (dinv, dinv_d), (W1b, W1b_d),
            (W2b, W2b_d), (W1f, W1f_d), (tW2f, tW2f_d), (tW1c, tW1c_d),
            (tb1c, tb1c_d), (tb2c, tb2c_d), (tcol, tcol_d), (epsc, eps_d),
            (idmat, idm_d), (onesr, ones_d),
        ]
        if not (bias1_zero and bias2_zero):
            loads_sync += [(sdrow, sd_d), (b1row, b1r_d), (b2row, b2r_d)]
        if not ln_trivial:
            loads_sync += [(lnwr, lnw_d), (lnbr, lnb_d)]
        N_LOADS = len(loads_sync)

        with nc.Block() as block:

            # ---------------- SYNC ----------------
            # One ordered stream of DMA work, merged by "trigger time" T on a
            # global window axis (conv1 windows 0..nt-1, conv2 nt..2nt-1) so
            # every wait's producer precedes it.
            @block.sync
            def _(sync):
                for dst_t, src_t in loads_sync:
                    sync.dma_start(dst_t[:, :], src_t[:, :]).then_inc(s_ld, 16)

                def ev_ag1(g):
                    wt(sync, s_dv_pa, TS * (g + 1))
                    r0 = g * TS * 128
                    sync.dma_start(
                        ag1_in[r0:r0 + TS * 128, :].rearrange(
                            "(w l) c -> l w c", l=128),
                        xw1[:, r0:r0 + TS * 128].rearrange(
                            "p (w c) -> p w c", c=128)).then_inc(s_tb1, 16)

                def ev_ag2(g):
                    wt(sync, s_dv_pa2, TS * (g + 1))
                    r0 = g * TS * 128
                    sync.dma_start(
                        ag2_in[r0:r0 + TS * 128, :].rearrange(
                            "(w l) c -> l w c", l=128),
                        xw2[:, r0:r0 + TS * 128].rearrange(
                            "p (w c) -> p w c", c=128)).then_inc(s_tb2, 16)

                def ev_oh(gg):
                    if gg >= NGRBUF:
                        rel = gg - NGRBUF
                        rc, rg = divmod(rel, NGRAN)
                        wt(sync, s_pe_run, rc * nt + gran_last_win[rg] + 1)
                    g = gg % NGRAN
                    i0 = g * GRAN
                    i1 = min(i0 + GRAN, nitem)
                    slot = gg % NGRBUF
                    sync.dma_start(
                        oh[:, slot * GRAN:slot * GRAN + (i1 - i0), :],
                        oh_d[:, i0 * 128:i1 * 128].rearrange(
                            "p (n e) -> p n e", e=128)).then_inc(s_ohs[slot], 16)

                def ev_out(g):
                    wt(sync, s_dv_drain, nt + TS * (g + 1))
                    r0 = g * TS * 128
                    sync.dma_start(
                        out_d[r0:r0 + TS * 128, :].rearrange(
                            "(w l) c -> l w c", l=128),
                        outst[:, g % NOUTS, :].rearrange(
                            "p (w c) -> p w c", c=128)).then_inc(s_outs[g % NOUTS], 16)

                events = []
                for g in range(NTILE):
                    events.append((-2, 0, ("ag1", g)))
                for gg in range(2 * NGRAN):
                    if gg < NGRBUF:
                        T = -1
                    else:
                        rel = gg - NGRBUF
                        rc, rg = divmod(rel, NGRAN)
                        T = rc * nt + gran_last_win[rg]
                    events.append((T, 1, ("oh", gg)))
                for g in range(NTILE):
                    # pa2 of window TS(g+1)-1 emitted around conv1 window
                    # (g+3)*TS-1 on vector's interleaved stream
                    T = min(nt - 1, (g + 3) * TS - 1)
                    events.append((T, 2, ("ag2", g)))
                for g in range(NTILE):
                    events.append((nt + TS * (g + 1) - 1, 3, ("out", g)))
                events.sort(key=lambda e: (e[0], e[1]))
                for _, _, (kind, arg) in events:
                    dict(ag1=ev_ag1, ag2=ev_ag2, oh=ev_oh, out=ev_out)[kind](arg)
                for g in range(NTILE):
                    wt(sync, s_outs[g % NOUTS], 16 * (g // NOUTS + 1))

            # ---------------- GPSIMD ----------------
            @block.gpsimd
            def _(gpsimd):
                gpsimd.load_library(mlp)
                for conv in range(2):
                    wt(gpsimd, s_tb1 if conv == 0 else s_tb2, 16 * NTILE)
                    gpsimd.collective_compute(
                        "AllGather", Alu.bypass,
                        replica_groups=[list(range(N_CORES))],
                        ins=[ag_ins[conv].ap().opt()],
                        outs=[tables[conv].ap().opt()]).then_inc(s_cc, 1)
                    wt(gpsimd, s_cc, conv + 1)
                    maxcall = max(ncall_q) if ncall_q else 0
                    sim_serial = bool(os.environ.get("BASS_SIM_SERIALIZE_GATHERS"))
                    for j in range(maxcall):
                        for q in range(NBUK):
                            if j >= ncall_q[q]:
                                continue
                            if sim_serial and (conv * ncall_q[q] + j) > 0:
                                # sim-only: SWDGE per-queue completion order is
                                # real on HW but not modeled by MultiCoreSim
                                wt(gpsimd, s_gq[q],
                                   16 * (conv * ncall_q[q] + j))
                            if j >= NGBQ:
                                rel_w = call_rel_win[q][j - NGBQ]
                                wt(gpsimd, s_pe_run, conv * nt + rel_w + 1)
                            nchk = call_sizes[q][j]
                            nidx = nchk * 128
                            ec0 = buk_base[q] + j * CALL_CHUNKS
                            gslot = (conv * ncall_q[q] + j) % NGBQ
                            dstap = gb[:, q, gslot, 0:nchk * 128].rearrange(
                                "p (n e) -> p n e", e=128)
                            gpsimd.dma_gather(
                                dstap,
                                tables[conv][q * bukrows:(q + 1) * bukrows, :],
                                gidx[:, ec0 * 8:(ec0 + nchk) * 8], nidx, nidx,
                                128, single_packet=True, queue_num=q,
                            ).then_inc(s_gq[q], 16)
                for q in range(NBUK):
                    if ncall_q[q]:
                        wt(gpsimd, s_gq[q], 16 * 2 * ncall_q[q])

            # ---------------- TENSOR ----------------
            @block.tensor
            def _(tensor):
                wt(tensor, s_ld, 16 * N_LOADS)
                # t-MLP
                wt(tensor, s_dv_tm, 1)
                tensor.matmul(ps_pa[0][:, 0:1], tW2f[:, :], ucol[:, :],
                              start=True, stop=True).then_inc(s_pe_tm, 1)
                wt(tensor, s_dv_tm, 2)
                tensor.matmul(ps_pa[1][0:1, 0:128], vcol[:, :], W1f[:, :],
                              start=True, stop=True).then_inc(s_pe_tm, 1)
                wt(tensor, s_dv_tm, 3)
                # conv1 phase A
                for t in range(nt):
                    if t >= 2:
                        wt(tensor, s_dv_pa, t - 1)
                    p = ps_pa[t % 2]
                    tensor.matmul(p[:, :], hsT[:, t * 128:(t + 1) * 128], W1b[:, :],
                                  start=True, stop=False)
                    tensor.matmul(p[:, :], onesr[:, :], r1bf[:, :],
                                  start=False, stop=True).then_inc(s_pe_pa, 1)

                def a2_mm(e):
                    wt(tensor, s_dv_trc, e + 1)
                    if e >= 2:
                        wt(tensor, s_dv_pa2, e - 1)
                    tensor.matmul(ps_pa[e % 2][:, :], h2T[:, e % 4, :],
                                  W2b[:, :], start=True,
                                  stop=True).then_inc(s_pe_pa2, 1)

                a2_state = {"e": 0}

                def tr_a2_upto(elim):
                    while a2_state["e"] < elim:
                        e = a2_state["e"]
                        wt(tensor, s_ac_h2, e // TS + 1)
                        if e >= 2:
                            wt(tensor, s_dv_trc, e - 1)
                        tensor.transpose(ps_tr[e % 2][:, :],
                                         xw1[:, e * 128:(e + 1) * 128],
                                         idmat[:, :]).then_inc(s_pe_tr, 1)
                        if e >= 1:
                            a2_mm(e - 1)
                        a2_state["e"] = e + 1

                def agg_windows(conv, xw, brow, bias_zero, interleave_a2):
                    for w in range(nt):
                        W = conv * nt + w
                        if W >= NPS:
                            wt(tensor, s_dv_drain, W - NPS + 1)
                        if conv == 1:
                            wt(tensor, s_dv_pa2, w + 1)
                        i0 = win_item0[w]
                        i1 = win_item0[w + 1] if w + 1 < nt else nitem
                        assert i1 > i0, f"window {w} has no edge chunks"
                        self_only = bool(os.environ.get("KDBG_SELF_ONLY"))
                        edge_only = bool(os.environ.get("KDBG_EDGE_ONLY"))
                        # self-loop: identity x local table slice
                        if not edge_only:
                            mm0 = tensor.matmul(ps_run[W % NPS][:, :], idmat[:, :],
                                          xw[:, w * 128:(w + 1) * 128],
                                          start=True, stop=self_only)
                            if self_only:
                                mm0.then_inc(s_pe_run, 1)
                        if not bias_zero:
                            tensor.matmul(ps_run[W % NPS][:, :],
                                          sdrow[:, w * 128:(w + 1) * 128],
                                          brow[:, :], start=False, stop=False)
                        for i in range(i0, i1):
                            it = items[i]
                            gg = conv * NGRAN + i // GRAN
                            wt(tensor, s_ohs[gg % NGRBUF], 16 * (gg // NGRBUF + 1))
                            wt(tensor, s_gq[it.q],
                               16 * (conv * ncall_q[it.q] +
                                     it.pos // CALL_CHUNKS + 1))
                            if self_only:
                                continue
                            slot = (conv * ncall_q[it.q] +
                                    it.pos // CALL_CHUNKS) % NGBQ
                            off = it.pos % CALL_CHUNKS
                            rhs = gb[:, it.q, slot, off * 128:(off + 1) * 128]
                            lhs = oh[:, (gg % NGRBUF) * GRAN + i % GRAN, :]
                            last = (i == i1 - 1)
                            mm = tensor.matmul(ps_run[W % NPS][:, :], lhs, rhs,
                                               start=(edge_only and i == i0),
                                               stop=last)
                            if last:
                                mm.then_inc(s_pe_run, 1)
                        if interleave_a2:
                            # transposes/A2 of tile t once tile t+1 drained
                            # (lag one tile so the LN chain has slack)
                            done_tiles = (w + 1) // TS - 1
                            if done_tiles > 0:
                                tr_a2_upto(min(nt, done_tiles * TS))

                agg_windows(0, xw1, b1row, bias1_zero, True)
                tr_a2_upto(nt)
                a2_mm(nt - 1)
                agg_windows(1, xw2, b2row, bias2_zero, False)

            # ---------------- VECTOR ----------------
            @block.vector
            def _(vector):
                wt(vector, s_ld, 16 * N_LOADS)
                # t-MLP
                vector.tensor_scalar(ucol[:, :], tW1c[:, :], tcol[:, :], tb1c[:, :],
                                     Alu.mult, Alu.add)
                vector.drain()
                vector.tensor_relu(ucol[:, :], ucol[:, :]).then_inc(s_dv_tm, 1)
                wt(vector, s_pe_tm, 1)
                vector.tensor_add(vcol[:, :], ps_pa[0][:, 0:1],
                                  tb2c[:, :]).then_inc(s_dv_tm, 1)
                wt(vector, s_pe_tm, 2)
                vector.tensor_copy(r1bf[:, :],
                                   ps_pa[1][0:1, 0:128]).then_inc(s_dv_tm, 1)
                # conv1 phase-A PSUM -> SBUF with dinv scale (bf16)
                for t in range(nt):
                    wt(vector, s_pe_pa, t + 1)
                    vector.tensor_scalar(xw1[:, t * 128:(t + 1) * 128],
                                         ps_pa[t % 2][:, :], dinv[:, t:t + 1],
                                         None, Alu.mult).then_inc(s_dv_pa, 1)

                def pa2_copy(e):
                    wt(vector, s_pe_pa2, e + 1)
                    vector.tensor_scalar(xw2[:, e * 128:(e + 1) * 128],
                                         ps_pa[e % 2][:, :], dinv[:, e:e + 1],
                                         None, Alu.mult).then_inc(s_dv_pa2, 1)

                trc_state = {"e": 0}

                def trc_pa2_upto(elim):
                    while trc_state["e"] < elim:
                        e = trc_state["e"]
                        wt(vector, s_pe_tr, e + 1)
                        if e >= 4:
                            wt(vector, s_pe_pa2, e - 3)
                        vector.tensor_copy(h2T[:, e % 4, :],
                                           ps_tr[e % 2][:, :]).then_inc(s_dv_trc, 1)
                        if e >= 1:
                            pa2_copy(e - 1)
                        trc_state["e"] = e + 1

                # conv1 drains + batched LN stats per tile, with trc/pa2 of
                # earlier tiles interleaved
                for g in range(NTILE):
                    for w in range(g * TS, (g + 1) * TS):
                        wt(vector, s_pe_run, w + 1)
                        vector.tensor_scalar(aggb[:, w * 128:(w + 1) * 128],
                                             ps_run[w % NPS][:, :],
                                             dinv[:, w:w + 1], None,
                                             Alu.mult).then_inc(s_dv_drain, 1)
                    c0, c1 = g * TS * 128, (g + 1) * TS * 128
                    gsl = slice(g * TS, (g + 1) * TS)
                    a3 = aggb[:, c0:c1].rearrange("p (n e) -> p n e", e=128)
                    vector.drain()
                    vector.tensor_mul(sqscr[:, :], aggb[:, c0:c1], aggb[:, c0:c1])
                    vector.tensor_reduce(ssum[:, gsl], a3, AxisX, Alu.add)
                    vector.drain()
                    vector.tensor_reduce(
                        ssq[:, gsl],
                        sqscr[:, :].rearrange("p (n e) -> p n e", e=128),
                        AxisX, Alu.add)
                    vector.tensor_scalar(nmu[:, gsl], ssum[:, gsl],
                                         -1.0 / 128.0, None, Alu.mult)
                    vector.drain()
                    vector.tensor_mul(varn[:, gsl], nmu[:, gsl], nmu[:, gsl])
                    vector.tensor_scalar(sq14[:, gsl], ssq[:, gsl],
                                         1.0 / 128.0, None, Alu.mult)
                    vector.drain()
                    vector.tensor_sub(varn[:, gsl], sq14[:, gsl],
                                      varn[:, gsl]).then_inc(s_dv_st, 1)
                    wt(vector, s_ac_sq, g + 1)
                    vector.reciprocal(rstd[:, gsl], sdev[:, gsl])
                    vector.drain()
                    vector.tensor_mul(nmurstd[:, gsl], nmu[:, gsl],
                                      rstd[:, gsl]).then_inc(s_dv_rs, 1)
                    vector.drain()
                    for w in range(g * TS, (g + 1) * TS):
                        vector.tensor_scalar(aggb[:, w * 128:(w + 1) * 128],
                                             aggb[:, w * 128:(w + 1) * 128],
                                             rstd[:, w:w + 1],
                                             nmurstd[:, w:w + 1], Alu.mult,
                                             Alu.add).then_inc(s_ac_z, 1)
                    if not ln_trivial:
                        wt(vector, s_ac_z, (g + 1) * TS)
                        vector.tensor_mul(aggb[:, c0:c1], aggb[:, c0:c1],
                                          lnwr[:, :])
                        vector.drain()
                        vector.tensor_add(aggb[:, c0:c1], aggb[:, c0:c1],
                                          lnbr[:, :]).then_inc(s_dv_st, 1)
                    trc_pa2_upto(g * TS)

                trc_pa2_upto(nt)
                pa2_copy(nt - 1)

                # conv2 drains into rotating out staging
                for g in range(NTILE):
                    if g >= NOUTS:
                        wt(vector, s_outs[g % NOUTS], 16 * ((g - NOUTS) // NOUTS + 1))
                    for w in range(g * TS, (g + 1) * TS):
                        wt(vector, s_pe_run, nt + w + 1)
                        vector.tensor_scalar(
                            outst[:, g % NOUTS, (w - g * TS) * 128:
                                  (w - g * TS + 1) * 128],
                            ps_run[(nt + w) % NPS][:, :], dinv[:, w:w + 1],
                            None, Alu.mult).then_inc(s_dv_drain, 1)

            # ---------------- SCALAR (ACT) ----------------
            @block.scalar
            def _(scalar):
                wt(scalar, s_ld, 16 * N_LOADS)
                for g in range(NTILE):
                    gsl = slice(g * TS, (g + 1) * TS)
                    wt(scalar, s_dv_st, g + 1 if ln_trivial else 2 * g + 1)
                    scalar.activation(sdev[:, gsl], varn[:, gsl], Act.Sqrt,
                                      bias=epsc[:, :]).then_inc(s_ac_sq, 1)
                    wt(scalar, s_ac_z, (g + 1) * TS)
                    if not ln_trivial:
                        wt(scalar, s_dv_st, 2 * g + 2)
                    scalar.drain()
                    c0, c1 = g * TS * 128, (g + 1) * TS * 128
                    scalar.activation(xw1[:, c0:c1], aggb[:, c0:c1],
                                      Act.Relu).then_inc(s_ac_h2, 1)

        nc.compile()
    return nc


# ---------------------------------------------------------------------------
# top level
# ---------------------------------------------------------------------------

LAST_NC = None


def _run_problem(h_noisy, edge_index, t, tW1, tb1, tW2, tb2, W1, b1, W2, b2,
                 ln_w, ln_b, n_nodes, shard, trace_dir=None):
    K = N_CORES
    npad = shard * K
    src = np.asarray(edge_index[0], np.int64)
    dst = np.asarray(edge_index[1], np.int64)

    deg = (np.bincount(dst, minlength=n_nodes).astype(np.float32) + 1.0)
    dinv = (1.0 / np.sqrt(deg)).astype(np.float32)
    dinv_pad = np.ones(npad, np.float32)
    dinv_pad[:n_nodes] = dinv
    sqdeg_pad = np.ones(npad, np.float32)
    sqdeg_pad[:n_nodes] = np.sqrt(deg)

    S = _make_schedule(src, dst, shard)
    nt = S["nt"]

    ln_trivial = bool(np.all(np.asarray(ln_w) == 1.0)
                      and np.all(np.asarray(ln_b) == 0.0))
    bias1_zero = bool(np.all(np.asarray(b1) == 0.0))
    bias2_zero = bool(np.all(np.asarray(b2) == 0.0))

    h_pad = np.zeros((npad, C), np.float32)
    h_pad[:n_nodes] = np.asarray(h_noisy, np.float32)

    shared = {
        "W1b": np.asarray(W1, np.float32).astype(BF16),
        "W2b": np.asarray(W2, np.float32).astype(BF16),
        "W1f": np.asarray(W1, np.float32),
        "tW2f": np.asarray(tW2, np.float32),
        "tW1col": np.asarray(tW1, np.float32).reshape(C, 1),
        "tb1col": np.asarray(tb1, np.float32).reshape(C, 1),
        "tb2col": np.asarray(tb2, np.float32).reshape(C, 1),
        "tcol": np.full((C, 1), np.float32(np.asarray(t).reshape(-1)[0]), np.float32),
        "epscol": np.full((128, 1), 1e-5, np.float32),
        "idmat": np.eye(128, dtype=np.float32).astype(BF16),
        "onesrow": np.ones((1, 128), np.float32).astype(BF16),
        "b1row": np.asarray(b1, np.float32).reshape(1, C).astype(BF16),
        "b2row": np.asarray(b2, np.float32).reshape(1, C).astype(BF16),
        "lnwrep": np.tile(np.asarray(ln_w, np.float32).reshape(1, C),
                          (128, TS)).astype(BF16),
        "lnbrep": np.tile(np.asarray(ln_b, np.float32).reshape(1, C),
                          (128, TS)).astype(BF16),
    }

    in_maps = []
    for k in range(K):
        gidx, oh = S["core_arrays"][k]
        hs = h_pad[k * shard:(k + 1) * shard].astype(BF16)
        dinvcol = np.zeros((128, nt), np.float32)
        for w in range(nt):
            dinvcol[:, w] = dinv_pad[k * shard + w * 128: k * shard + (w + 1) * 128]
        m = dict(shared)
        m["h_sT"] = np.ascontiguousarray(hs.T)
        m["gidx"] = gidx
        m["ohmat"] = oh
        m["dinvcol"] = dinvcol
        m["sqdegrow"] = sqdeg_pad[k * shard:(k + 1) * shard].reshape(1, shard).astype(BF16)
        in_maps.append(m)

    nc = _build(S, shard, ln_trivial, bias1_zero, bias2_zero)
    global LAST_NC
    LAST_NC = nc

    if trace_dir is not None:
        res = _run_traced(nc, in_maps, trace_dir)
    else:
        res = run_bass_kernel_spmd(nc, in_maps, list(range(K)))

    out = np.concatenate([res.results[k]["out_shard"] for k in range(K)], axis=0)
    return out[:n_nodes].astype(np.float32)


def _run_traced(nc, in_maps, trace_dir):
    """Run with NRT/NTFF profiling via the axon ctypes hook (test harness)."""
    import types
    import antenv
    if "antenv.axon_hooks" not in sys.modules:
        mod = types.ModuleType("antenv.axon_hooks")
        mod._hook = None
        mod.set_axon_ntff_profile_hook = lambda h: setattr(mod, "_hook", h)
        mod.get_axon_ntff_profile_hook = lambda: mod._hook
        sys.modules["antenv.axon_hooks"] = mod
        antenv.axon_hooks = mod
    from trn_agent_boot.trn_boot import _ntff_profile_via_ctypes
    hook = _ntff_profile_via_ctypes("/opt/axon/libaxon_pjrt.so")
    os.makedirs(trace_dir, exist_ok=True)
    with hook(trace_dir, [0]):
        res = run_bass_kernel_spmd(nc, in_maps, list(range(N_CORES)))
    return res


def kernel(h_noisy, edge_index, t, tW1, tb1, tW2, tb2, W1, b1, W2, b2,
           ln_w, ln_b):
    trace_dir = os.environ.get("BASS_KERNEL_TRACE_DIR") or None
    return _run_problem(
        np.asarray(h_noisy), np.asarray(edge_index), np.asarray(t),
        np.asarray(tW1), np.asarray(tb1), np.asarray(tW2), np.asarray(tb2),
        np.asarray(W1), np.asarray(b1), np.asarray(W2), np.asarray(b2),
        np.asarray(ln_w), np.asarray(ln_b),
        n_nodes=N_NODES, shard=12544, trace_dir=trace_dir)
